# revision 68
# baseline (speedup 1.0000x reference)
"""Trainium2 Bass kernel for nn_LASLNNet (complex-valued 4D CNN).

Strategy (8 NeuronCores, SPMD single program):
  - core c handles (batch b = c//2, spatial half h = c%2) -> 4 x 2 split.
  - All complex convs are computed as real matmuls with doubled channels:
      [yr; yi] = [[Wr, Wi], [-Wi, Wr]]^T @ [xr; xi]
  - Every matmul keeps K uniform (the PE array reconfigures between
    different contraction sizes at a ~2.3x throughput penalty, so K=64
    slots are padded / packed rather than issued as narrow matmuls).
  - conv1 (k=3,s=2): im2col slabs on host (27 (j1,j2,j3) tap slabs,
    K=55 incl. a bias/ones row); j4 handled as 3 PSUM-accumulated
    K=64 matmuls with step-2 rhs reads. M=128: output channels are
    duplicated in the weight columns so PSUM partitions 64..127 carry a
    second copy used to build shifted x2 replicas without DMA.
  - x2 store: d4-padded flat grid [block(d1) 7, d2 9, d3 9, d4 10];
    x2t partitions 0..63 hold x2, partitions 64..127 hold x2 shifted +1
    (written by a second activation with dst offset -1, pad columns
    supplied by the initial memset). A second tile x2s810 holds x2 on
    partitions 0..63 (DVE block copies) and x2 shifted +810 = one d1
    block on partitions 64..127 (third activation per conv1 row).
  - conv2 (k=3,s=1,p=1): per (row, o2-group) PSUM accumulation of
    45 K=128 matmuls: 27 (j1,j2,j3) taps with j4 in {0,1} fused via the
    +1 replica; 9 (j2,j3) taps with j4=2, j1 in {0,1} fused via the
    +810 replica; j1=2, j4=2 taps: per j2, (j3=0,j3=1) fused via a +10
    replica (x2s10, built like x2s8) plus an o3=0 strip and the j3=2
    single with zero upper-half weights. Edge taps restrict (o2,o3)
    ranges via strided APs; PSUM has_written semantics make
    partial-region accumulation correct (the first matmul of each group
    is the full-region interior tap). w2a is packed in tap-emission
    order and DMA'd in two pieces so early matmuls aren't gated on the
    full transfer.
  - conv3/4/5 (1x1): plain matmuls on a row-aligned layout, interleaved
    into the conv2 row stream 1-2 rows behind so their inputs'
    activations are drained before the PE reaches them. Activations
    alternate between the Scalar and Vector engines.
  - FC: on-chip mul+reduce against host-sliced fcw; final cross-half
    sum + fc bias on host (each core returns a [128,1] partial).
  - dtype: bf16 matmul operands, fp32 PSUM/copies.

Spatial split along first output spatial dim D1 (9 rows), dup-free:
  half 0 -> conv2..4 rows 0..3 + row 4's o2 in [0,5); half 1 sees
  d1/d2-mirrored inputs and j1/j2-flipped conv weights, so the same
  program computes global rows 5..8 + row 4's o2 in (4,8] in its local
  coordinates. conv1 row 0 is a dummy zero row on every core (supplied
  by memset, never computed). The o2=4 overlap column and each half's
  foreign row-4 tail are masked via the per-core fcw slice.
"""

import itertools

import numpy as np
import ml_dtypes

import concourse.bacc as bacc
import concourse.mybir as mybir
from concourse.tile import TileContext
from concourse.bass_utils import run_bass_kernel_spmd

F32 = mybir.dt.float32
BF16 = mybir.dt.bfloat16
BF = ml_dtypes.bfloat16

NB = 4            # batch
R1 = 7            # conv1 rows computed per core (incl. dummy edge rows)
R2 = 5            # conv2/3/4 rows per core
R5 = 3            # conv5 rows per core
D4P = 10          # d4-padded inner dim (9 valid + 1 zero)
BLK = 9 * 9 * D4P                # 810, one d1-block of x2
X2N = R1 * BLK                   # logical x2 elements per partition
S1R = 9 * 9 * 20                 # 1620, conv1 slab elements per row
S1N = R1 * S1R                   # 11340 conv1 slab elements per partition
N3 = R2 * 729                    # 3645 compact columns for conv3/4
N5 = R5 * 125                    # 375 conv5 output columns

# conv2 (j1,j2,j3) taps in emission order (interior first for PSUM
# has_written coverage); w2a is packed in this order so an early partial
# DMA covers the first matmuls of each group.
TAPS27 = sorted(itertools.product(range(3), repeat=3),
                key=lambda t: (t != (1, 1, 1)))

_CACHE = {}


def _build_nc():
    nc = bacc.Bacc("TRN2", target_bir_lowering=False, debug=False)

    # Inputs are packed into few tensors: each dma_start adds ~620ns to
    # the end-of-NEFF queue drain, so fewer/larger transfers win.
    x1_d = nc.dram_tensor("x1", [64, S1N], BF16, kind="ExternalInput")
    w1_d = nc.dram_tensor("w1", [64, 3 * 128], BF16, kind="ExternalInput")
    w2a_d = nc.dram_tensor("w2a", [128, 27 * 128], BF16, kind="ExternalInput")
    # w2cd: [w2c 1152 | w2d 1152 | w2e 384]
    w2cd_d = nc.dram_tensor("w2cd", [128, 2688], BF16, kind="ExternalInput")
    # w345: [w3 256 | w4 512 | w5 256]
    w345_d = nc.dram_tensor("w345", [128, 1024], BF16, kind="ExternalInput")
    # fpack: [b2 1 | b3 2 | b4 2 | b5 1 | fcw 375]
    fpk_d = nc.dram_tensor("fpk", [128, 381], F32, kind="ExternalInput")
    out_d = nc.dram_tensor("out", [128, 1], F32, kind="ExternalOutput")

    Relu = mybir.ActivationFunctionType.Relu

    with TileContext(nc) as tc:
        with tc.tile_pool(name="sb", bufs=1) as pool, \
             tc.tile_pool(name="ps", bufs=8, space="PSUM") as pp:
            x1t = pool.tile([64, S1N], BF16, tag="x1")
            w1t = pool.tile([64, 3 * 128], BF16, tag="w1")
            # x2 store: [1 lead margin][R1 blocks of BLK][1 tail margin]
            x2t = pool.tile([128, X2N + 92], BF16, tag="x2")
            x2s8 = pool.tile([128, X2N + 92], BF16, tag="x2s8")
            x2s10 = pool.tile([128, X2N + 92], BF16, tag="x2s10")
            w2at = pool.tile([128, 27 * 128], BF16, tag="w2a")
            w2cdt = pool.tile([128, 2688], BF16, tag="w2cd")
            w345t = pool.tile([128, 1024], BF16, tag="w345")
            fpkt = pool.tile([128, 381], F32, tag="fpk")
            x3t = pool.tile([128, N3], BF16, tag="x3")
            x4t = pool.tile([128, 2 * N3], BF16, tag="x4")
            x4bt = pool.tile([128, 2 * N3], BF16, tag="x4b")
            x5t = pool.tile([128, N5], F32, tag="x5")
            prodt = pool.tile([128, N5], F32, tag="prod")
            fct = pool.tile([128, 1], F32, tag="fc")
            w2ct = w2cdt[:, 0:1152]
            w2dt = w2cdt[:, 1152:2304]
            w2et = w2cdt[:, 2304:2688]
            w3t = w345t[:, 0:256]
            w4t = w345t[:, 256:768]
            w5t = w345t[:, 768:1024]
            b2t = fpkt[:, 0:1]
            b3t = fpkt[:, 1:3]
            b4t = fpkt[:, 3:5]
            b5t = fpkt[:, 5:6]
            fcwt = fpkt[:, 6:381]

            # weights first (small w1 unblocks conv1), then x1 row chunks
            # so conv1 row r only waits for its own slab chunk.
            # slab row 0 is the dummy zero row on every core (mirror
            # symmetry) -> conv1 row 0 is never computed, never DMA'd
            nc.sync.dma_start(w1t[:, :], w1_d[:, :])
            for (ra, rb) in ((1, 3), (3, 5)):
                nc.sync.dma_start(x1t[:, ra * S1R:rb * S1R],
                                  x1_d[:, ra * S1R:rb * S1R])
            # first 9 tap blocks of w2a land before x1's last chunk: the
            # PE only reaches conv1 rows 5-6 ~4us after it needs conv2
            # row 0's first taps, so this ordering closes the PE gap at
            # the conv1 -> conv2 transition.
            nc.sync.dma_start(w2at[:, 0:9 * 128], w2a_d[:, 0:9 * 128])
            nc.sync.dma_start(x1t[:, 5 * S1R:7 * S1R],
                              x1_d[:, 5 * S1R:7 * S1R])
            nc.sync.dma_start(w2at[:, 9 * 128:], w2a_d[:, 9 * 128:])
            nc.sync.dma_start(w2cdt[:, :], w2cd_d[:, :])
            nc.sync.dma_start(w345t[:, :], w345_d[:, :])
            nc.sync.dma_start(fpkt[:, :], fpk_d[:, :])

            # ---------------- conv1 ----------------
            # slab view: [r(R1), o2(9), o3(9), d4(20)]
            s1v = x1t.rearrange("p (r a b c) -> p r a b c", r=R1, a=9, b=9, c=20)
            # x2 logical views. Lower (alloc offset 1): plain x2.
            x2v = x2t[:, 1:1 + X2N].rearrange(
                "p (r a b c) -> p r a b c", r=R1, a=9, b=9, c=D4P)
            # Upper of x2t (alloc offset 0): x2 shifted +1; the d4 pad
            # column of each cell is never written -> stays 0 from the
            # memset, which is exactly value x2[pad]=0 shifted into place.
            x2u = x2t[:, 0:X2N].rearrange(
                "p (r a b c) -> p r a b c", r=R1, a=9, b=9, c=D4P)
            # Upper of x2s8 (alloc offset 1): x2 shifted +810 (one block):
            # conv1 row r output is written at block slot r-1.
            x2s8u = x2s8[:, 1:1 + X2N].rearrange(
                "p (r a b c) -> p r a b c", r=R1, a=9, b=9, c=D4P)

            # Zero the grids (pad columns, margins, unwritten gaps). Must
            # cover full cells, not just the k=9 pads: a pads-only memset
            # is region-disjoint from the activations, so nothing would
            # order it before them, and k=8/k=9 share a 4-byte word ->
            # engine write race. Overlapping regions serialize via Tile
            # deps; gpsimd runs these during the x1 DMA, off the critical
            # path. x2s8's lower half needs no zeroing (DVE block copies
            # are its only writer and cover every read).
            nc.gpsimd.memset(x2t[0:64, 0:X2N + 1], 0)
            nc.vector.memset(x2t[64:128, 0:X2N], 0)
            nc.vector.memset(x2s8[64:128, 1:1 + X2N], 0)
            nc.gpsimd.memset(x2s10[64:128, 0:1 + X2N], 0)
            # block 0 of x2s8's lower half is the dummy zero block (no
            # conv1 row 0 -> no DVE copy writes it)
            nc.gpsimd.memset(x2s8[0:64, 1:1 + BLK], 0)
            # conv5 row 2 reads the (never-computed) o2 tail of x4b row 4;
            # zero it (1-col overlap with the conv4 act region orders the
            # memset before the act, avoiding a shared-word write race).
            for mh in range(2):
                t0c = mh * N3 + 4 * 729 + 404
                nc.gpsimd.memset(x4bt[:, t0c:(mh + 1) * N3], 0)

            def conv1_row(r):
                for (o2s, c2g) in ((0, 5), (5, 4)):
                    n = c2g * 81
                    ps1 = pp.tile([128, 512], F32, tag="ps")
                    ps1v = ps1[:, :n].rearrange("p (a b c) -> p a b c",
                                                a=c2g, b=9, c=9)
                    for j4 in range(3):
                        rhs = s1v[:, r, o2s:o2s + c2g, :, j4:j4 + 17:2]
                        nc.tensor.matmul(
                            ps1v[:, :, :, :],
                            w1t[:, j4 * 128:(j4 + 1) * 128],
                            rhs,
                            start=(j4 == 0), stop=(j4 == 2))
                    nc.scalar.activation(
                        x2v[0:64, r, o2s:o2s + c2g, :, 0:9],
                        ps1v[0:64, :, :, :],
                        Relu)
                    nc.vector.tensor_relu(
                        x2u[64:128, r, o2s:o2s + c2g, :, 0:9],
                        ps1v[64:128, :, :, :])
                    nc.scalar.activation(
                        x2s8u[64:128, r - 1, o2s:o2s + c2g, :, 0:9],
                        ps1v[64:128, :, :, :],
                        Relu)
                    if r >= 2:
                        # x2s10 upper: x2 shifted +10 (one d3 cell); row
                        # r's block lands 10 columns early.
                        s10d = x2s10[64:128, r * BLK - 9:r * BLK - 9
                                     + BLK].rearrange(
                            "p (a b c) -> p a b c", a=9, b=9, c=D4P)
                        nc.scalar.activation(
                            s10d[:, o2s:o2s + c2g, :, 0:9],
                            ps1v[64:128, :, :, :],
                            Relu)
                # lower halves of x2s8/x2s10: plain copies of the finished
                # block (same partitions -> DVE; covers pad columns)
                if r < 5:
                    nc.vector.tensor_copy(
                        x2s8[0:64, 1 + r * BLK:1 + (r + 1) * BLK],
                        x2t[0:64, 1 + r * BLK:1 + (r + 1) * BLK])
                if r >= 2:
                    nc.vector.tensor_copy(
                        x2s10[0:64, 1 + r * BLK:1 + (r + 1) * BLK],
                        x2t[0:64, 1 + r * BLK:1 + (r + 1) * BLK])

            # ---------------- conv2 ----------------
            taps = TAPS27
            taps9 = list(itertools.product(range(3), repeat=2))
            x3v = x3t.rearrange("p (r a b c) -> p r a b c", r=R2, a=9, b=9, c=9)

            def conv2_row(r):
                # row 4 is split between the halves along o2 (each half in
                # its own mirrored coords computes the (0,5) group only;
                # the o2=4 overlap column is de-duplicated via fcw masks)
                for (o2s, c2g) in (((0, 5), (5, 4)) if r < 4 else ((0, 5),)):
                    n = c2g * 81
                    ps2 = pp.tile([128, 512], F32, tag="ps")
                    ps2v = ps2[:, :n].rearrange("p (a b c) -> p a b c",
                                                a=c2g, b=9, c=9)

                    def region(j2, j3):
                        lo2 = max(o2s, 1 - j2)
                        hi2 = min(o2s + c2g, 10 - j2)
                        lo3 = max(0, 1 - j3)
                        hi3 = min(9, 10 - j3)
                        return lo2, hi2, lo3, hi3

                    # 27 (j1,j2,j3) taps, j4 in {0,1} via the +1 replica
                    for ti, (j1, j2, j3) in enumerate(taps):
                        lo2, hi2, lo3, hi3 = region(j2, j3)
                        c2 = hi2 - lo2
                        c3 = hi3 - lo3
                        out_ap = ps2v[:, lo2 - o2s:hi2 - o2s, lo3:hi3, :]
                        t27 = ti
                        # alloc base for (o2=lo2, o3=lo3, o4=0), j4=0 on the
                        # base partitions (the +1 alloc offset and the -1
                        # d4 pad shift cancel):
                        base0 = ((r + j1) * BLK + (lo2 + j2 - 1) * 90
                                 + (lo3 + j3 - 1) * D4P)
                        rhs0 = x2t[:, base0:base0 + c2 * 90].rearrange(
                            "p (a b c) -> p a b c", a=c2, b=9, c=D4P)[
                            :, :, 0:c3, 0:9]
                        nc.tensor.matmul(
                            out_ap,
                            w2at[:, t27 * 128:(t27 + 1) * 128],
                            rhs0,
                            start=(ti == 0), stop=False)
                    # 9 (j2,j3) taps, j4=2, j1 in {0,1} via the +810 replica
                    for t9, (j2, j3) in enumerate(taps9):
                        lo2, hi2, lo3, hi3 = region(j2, j3)
                        c2 = hi2 - lo2
                        c3 = hi3 - lo3
                        out_ap = ps2v[:, lo2 - o2s:hi2 - o2s, lo3:hi3, :]
                        base = (r * BLK + (lo2 + j2 - 1) * 90
                                + (lo3 + j3 - 1) * D4P + 2)
                        rhs = x2s8[:, base:base + c2 * 90].rearrange(
                            "p (a b c) -> p a b c", a=c2, b=9, c=D4P)[
                            :, :, 0:c3, 0:9]
                        nc.tensor.matmul(
                            out_ap,
                            w2ct[:, t9 * 128:(t9 + 1) * 128],
                            rhs,
                            start=False, stop=False)
                    # j4=2, j1=2 taps: per j2, the (j3=0, j3=1) pair rides
                    # the +10 replica over o3 in [1,9) (both taps full
                    # there); j3=1's o3=0 column and the j3=2 tap remain
                    # as zero-upper singles.
                    for j2 in range(3):
                        lo2, hi2, _, _ = region(j2, 0)
                        c2 = hi2 - lo2
                        out_ap = ps2v[:, lo2 - o2s:hi2 - o2s, 1:9, :]
                        base = ((r + 2) * BLK + (lo2 + j2 - 1) * 90 + 2)
                        rhs = x2s10[:, base:base + c2 * 90].rearrange(
                            "p (a b c) -> p a b c", a=c2, b=9, c=D4P)[
                            :, :, 0:8, 0:9]
                        nc.tensor.matmul(
                            out_ap,
                            w2et[:, j2 * 128:(j2 + 1) * 128],
                            rhs,
                            start=False, stop=False)
                        # strip: (j1=2, j2, j3=1, j4=2) at o3=0
                        strip_ap = ps2v[:, lo2 - o2s:hi2 - o2s, 0:1, :]
                        sbase = ((r + 2) * BLK + (lo2 + j2 - 1) * 90 + 2)
                        srhs = x2t[:, sbase:sbase + c2 * 90].rearrange(
                            "p (a b c) -> p a b c", a=c2, b=9, c=D4P)[
                            :, :, 0:1, 0:9]
                        nc.tensor.matmul(
                            strip_ap,
                            w2dt[:, (j2 * 3 + 1) * 128:(j2 * 3 + 2) * 128],
                            srhs,
                            start=False, stop=False)
                        # single: (j1=2, j2, j3=2, j4=2)
                        lo2, hi2, lo3, hi3 = region(j2, 2)
                        c2 = hi2 - lo2
                        c3 = hi3 - lo3
                        out_ap = ps2v[:, lo2 - o2s:hi2 - o2s, lo3:hi3, :]
                        base = ((r + 2) * BLK + (lo2 + j2 - 1) * 90
                                + (lo3 + 1) * D4P + 2)
                        rhs = x2t[:, base:base + c2 * 90].rearrange(
                            "p (a b c) -> p a b c", a=c2, b=9, c=D4P)[
                            :, :, 0:c3, 0:9]
                        nc.tensor.matmul(
                            out_ap,
                            w2dt[:, (j2 * 3 + 2) * 128:(j2 * 3 + 3) * 128],
                            rhs,
                            start=False, stop=(j2 == 2))
                    nc.scalar.activation(
                        x3v[:, r, o2s:o2s + c2g, :, :],
                        ps2v[:, :, :, :],
                        Relu, bias=b2t[:, :])

            # ---------------- conv3/4/5 per-row emitters ----------------
            # Row-aligned chunks so a row's 1x1 convs can ride right
            # behind the producing activations (interleaved below).
            rchunks = ((0, 405), (405, 324))

            def conv3_row(r):
                rch = rchunks if r < 4 else rchunks[:1]
                for (pos, sz) in rch:
                    for mh in range(2):
                        p0 = r * 729 + pos
                        ps3 = pp.tile([128, 512], F32, tag="ps")
                        nc.tensor.matmul(
                            ps3[:, :sz],
                            w3t[:, mh * 128:(mh + 1) * 128],
                            x3t[:, p0:p0 + sz],
                            start=True, stop=True)
                        if mh == 0:
                            nc.scalar.activation(
                                x4t[:, mh * N3 + p0:mh * N3 + p0 + sz],
                                ps3[:, :sz],
                                Relu, bias=b3t[:, mh:mh + 1])
                        else:
                            nc.vector.tensor_scalar(
                                x4t[:, mh * N3 + p0:mh * N3 + p0 + sz],
                                ps3[:, :sz],
                                b3t[:, mh:mh + 1], 0.0,
                                mybir.AluOpType.add, mybir.AluOpType.max)

            def conv4_row(r):
                rch = rchunks if r < 4 else rchunks[:1]
                for (pos, sz) in rch:
                    for mh in range(2):
                        p0 = r * 729 + pos
                        ps4 = pp.tile([128, 512], F32, tag="ps")
                        nc.tensor.matmul(
                            ps4[:, :sz],
                            w4t[:, (mh * 2) * 128:(mh * 2 + 1) * 128],
                            x4t[:, p0:p0 + sz],
                            start=True, stop=False)
                        nc.tensor.matmul(
                            ps4[:, :sz],
                            w4t[:, (mh * 2 + 1) * 128:(mh * 2 + 2) * 128],
                            x4t[:, N3 + p0:N3 + p0 + sz],
                            start=False, stop=True)
                        if mh == 0:
                            nc.scalar.activation(
                                x4bt[:, mh * N3 + p0:mh * N3 + p0 + sz],
                                ps4[:, :sz],
                                Relu, bias=b4t[:, mh:mh + 1])
                        else:
                            nc.vector.tensor_scalar(
                                x4bt[:, mh * N3 + p0:mh * N3 + p0 + sz],
                                ps4[:, :sz],
                                b4t[:, mh:mh + 1], 0.0,
                                mybir.AluOpType.add, mybir.AluOpType.max)

            # x4b view: [mb(2), r(R2), o2(9), o3(9), o4(9)]
            x4bv = x4bt.rearrange("p (m r a b c) -> p m r a b c",
                                  m=2, r=R2, a=9, b=9, c=9)

            def conv5_row(rr):
                ps5 = pp.tile([128, 512], F32, tag="ps")
                for mb in range(2):
                    rhs = x4bv[:, mb, 2 * rr, 0:9:2, 0:9:2, 0:9:2]
                    nc.tensor.matmul(
                        ps5[:, :125],
                        w5t[:, mb * 128:(mb + 1) * 128],
                        rhs,
                        start=(mb == 0), stop=(mb == 1))
                nc.scalar.activation(
                    x5t[:, rr * 125:(rr + 1) * 125],
                    ps5[:, :125],
                    Relu, bias=b5t[:, :])

            # Interleaved schedule: conv1 rows feed conv2 rows two blocks
            # ahead; downstream layers lag far enough that their input
            # activations are already drained when the PE reaches them,
            # so the tensor engine never stalls.
            conv1_row(1)
            conv1_row(2)
            conv1_row(3)
            conv1_row(4)
            conv1_row(5)
            conv2_row(0)
            conv1_row(6)
            conv2_row(1)
            conv3_row(0)
            conv2_row(2)
            conv3_row(1)
            conv4_row(0)
            conv2_row(3)
            conv3_row(2)
            conv4_row(1)
            conv2_row(4)
            conv3_row(3)
            conv4_row(2)
            conv5_row(0)
            conv3_row(4)
            conv4_row(3)
            conv5_row(1)
            conv4_row(4)
            conv5_row(2)

            # ---------------- FC partials ----------------
            nc.vector.tensor_mul(prodt[:, :], x5t[:, :], fcwt[:, :])
            nc.vector.reduce_sum(fct[:, :], prodt[:, :],
                                 axis=mybir.AxisListType.X)

            nc.sync.dma_start(out_d[:, :], fct[:, :])

    nc.compile()
    return nc


# ---------------- host-side data prep ----------------

def _cplx_block(wr_t, wi_t):
    """[32ci r; 32ci i] x [64co r | 64co i] real-matmul block."""
    f32 = np.float32
    B = np.zeros((64, 128), f32)
    B[0:32, 0:64] = wr_t
    B[0:32, 64:128] = wi_t
    B[32:64, 0:64] = -wi_t
    B[32:64, 64:128] = wr_t
    return B


def _prep_conv12_weights(inputs, h):
    """w1/w2a/w2c/w2d for one half. h=1 cores see d1/d2-mirrored inputs,
    so their conv taps are flipped along j1 and j2."""
    f32 = np.float32
    fl = slice(None) if h == 0 else slice(None, None, -1)
    w1r = np.asarray(inputs["w1r"], f32)[:, 0][:, fl, fl]   # [32, 3,3,3,3]
    w1i = np.asarray(inputs["w1i"], f32)[:, 0][:, fl, fl]
    # [t27, j4, co]
    w1r_t = w1r.transpose(1, 2, 3, 4, 0).reshape(27, 3, 32)
    w1i_t = w1i.transpose(1, 2, 3, 4, 0).reshape(27, 3, 32)
    # [64, 3*128]: per j4 block, M=128 with duplicated 64-wide halves so
    # PSUM partitions 64..127 carry a copy (used for shifted replicas).
    W1 = np.zeros((64, 3 * 128), f32)
    for j4 in range(3):
        blk = np.zeros((64, 64), f32)
        blk[0:27, 0:32] = w1r_t[:, j4]
        blk[0:27, 32:64] = w1i_t[:, j4]
        blk[27:54, 0:32] = -w1i_t[:, j4]
        blk[27:54, 32:64] = w1r_t[:, j4]
        if j4 == 0:
            blk[54, 0:32] = np.asarray(inputs["b1r"], f32)
            blk[54, 32:64] = np.asarray(inputs["b1i"], f32)
        W1[:, j4 * 128:j4 * 128 + 64] = blk
        W1[:, j4 * 128 + 64:(j4 + 1) * 128] = blk

    w2r = np.asarray(inputs["w2r"], f32)[:, :, fl, fl]   # [64, 32, 3,3,3,3]
    w2i = np.asarray(inputs["w2i"], f32)[:, :, fl, fl]
    # [j1, j2, j3, j4, ci, co]
    w2r_t = w2r.transpose(2, 3, 4, 5, 1, 0)
    w2i_t = w2i.transpose(2, 3, 4, 5, 1, 0)
    # w2a: 27 (j1,j2,j3) taps in emission order,
    # rows 0:64 = j4=0, rows 64:128 = j4=1
    W2a = np.zeros((128, 27 * 128), f32)
    for t, (j1, j2, j3) in enumerate(TAPS27):
        W2a[0:64, t * 128:(t + 1) * 128] = _cplx_block(
            w2r_t[j1, j2, j3, 0], w2i_t[j1, j2, j3, 0])
        W2a[64:128, t * 128:(t + 1) * 128] = _cplx_block(
            w2r_t[j1, j2, j3, 1], w2i_t[j1, j2, j3, 1])
    # w2c: 9 (j2,j3) taps at j4=2, rows 0:64 = j1=0, rows 64:128 = j1=1
    W2c = np.zeros((128, 9 * 128), f32)
    W2d = np.zeros((128, 9 * 128), f32)
    for t, (j2, j3) in enumerate(itertools.product(range(3), repeat=2)):
        W2c[0:64, t * 128:(t + 1) * 128] = _cplx_block(
            w2r_t[0, j2, j3, 2], w2i_t[0, j2, j3, 2])
        W2c[64:128, t * 128:(t + 1) * 128] = _cplx_block(
            w2r_t[1, j2, j3, 2], w2i_t[1, j2, j3, 2])
        W2d[0:64, t * 128:(t + 1) * 128] = _cplx_block(
            w2r_t[2, j2, j3, 2], w2i_t[2, j2, j3, 2])
    # w2e: per j2, the (j1=2, j4=2) taps j3=0 (rows 0:64) and j3=1
    # (rows 64:128) paired via the +10 replica
    W2e = np.zeros((128, 3 * 128), f32)
    for j2 in range(3):
        W2e[0:64, j2 * 128:(j2 + 1) * 128] = _cplx_block(
            w2r_t[2, j2, 0, 2], w2i_t[2, j2, 0, 2])
        W2e[64:128, j2 * 128:(j2 + 1) * 128] = _cplx_block(
            w2r_t[2, j2, 1, 2], w2i_t[2, j2, 1, 2])
    return W1.astype(BF), W2a, W2c, W2d, W2e


def _prep_weights(inputs):
    f32 = np.float32
    B2 = np.concatenate([np.asarray(inputs["b2r"], f32),
                         np.asarray(inputs["b2i"], f32)])[:, None]

    w3r = np.asarray(inputs["w3r"], f32).reshape(128, 64)
    w3i = np.asarray(inputs["w3i"], f32).reshape(128, 64)
    W3 = np.zeros((128, 2 * 128), f32)
    W3[0:64, 0:128] = w3r.T
    W3[64:128, 0:128] = -w3i.T
    W3[0:64, 128:256] = w3i.T
    W3[64:128, 128:256] = w3r.T
    B3 = np.stack([np.asarray(inputs["b3r"], f32),
                   np.asarray(inputs["b3i"], f32)], axis=1)

    w4r = np.asarray(inputs["w4r"], f32).reshape(128, 128)
    w4i = np.asarray(inputs["w4i"], f32).reshape(128, 128)
    W4 = np.zeros((128, 4 * 128), f32)
    W4[:, 0:128] = w4r.T
    W4[:, 128:256] = -w4i.T
    W4[:, 256:384] = w4i.T
    W4[:, 384:512] = w4r.T
    B4 = np.stack([np.asarray(inputs["b4r"], f32),
                   np.asarray(inputs["b4i"], f32)], axis=1)

    w5r = np.asarray(inputs["w5r"], f32).reshape(64, 128)
    w5i = np.asarray(inputs["w5i"], f32).reshape(64, 128)
    W5 = np.zeros((128, 2 * 128), f32)
    W5[:, 0:64] = w5r.T
    W5[:, 64:128] = w5i.T
    W5[:, 128:192] = -w5i.T
    W5[:, 192:256] = w5r.T
    B5 = np.concatenate([np.asarray(inputs["b5r"], f32),
                         np.asarray(inputs["b5i"], f32)])[:, None]

    return (W3, W4, W5), np.concatenate([B2, B3, B4, B5], axis=1)


def _mirror_x(x_b):
    """d1/d2 double mirror of one batch's [20,20,20,20] input: only the
    windows [0,18] feed the convs, so mirror those and leave index 19."""
    xf = np.zeros_like(x_b)
    xf[0:19, 0:19] = x_b[18::-1, 18::-1]
    return xf


def _prep_x1(xr_b, xi_b):
    """Conv1 input slab (half-0 geometry; half 1 passes mirrored x):
    [64, R1, 9, 9, 20] bf16 with a dummy zero row at block 0."""
    S = np.zeros((64, R1, 9, 9, 20), np.float32)
    for t, (j1, j2, j3) in enumerate(itertools.product(range(3), repeat=3)):
        subr = xr_b[j1:j1 + 17:2, j2:j2 + 17:2, j3:j3 + 17:2, :]
        subi = xi_b[j1:j1 + 17:2, j2:j2 + 17:2, j3:j3 + 17:2, :]
        S[t, 1:7] = subr[0:6]
        S[27 + t, 1:7] = subi[0:6]
    S[54, 1:7] = 1.0
    return S.reshape(64, S1N).astype(BF)


def _prep_fcw(fcw, h):
    """Per-core fcw slice in local x5 column order. Local col
    (rr, i2, i3, i4) maps to global conv5 output (o1h, o2h, o3h, o4h) =
    (rr, i2, i3, i4) for h=0 and (4-rr, 4-i2, i3, i4) for h=1 (d1/d2
    mirror). Masks: h=0 owns the (o1h=2, o2h=2) overlap column; each
    half's row-4 o2 tail (i2 beyond its split) is garbage/foreign."""
    out = np.zeros((128, N5), np.float32)
    f = np.asarray(fcw, np.float32).reshape(5, 5, 5, 5)
    for rr in range(R5):
        for i2 in range(5):
            if rr == 2 and i2 >= (3 if h == 0 else 2):
                continue
            g = f[rr, i2] if h == 0 else f[4 - rr, 4 - i2]
            cols = rr * 125 + i2 * 25 + np.arange(25)
            out[:, cols] = g.reshape(-1)[None, :]
    return out


def _make_in_maps(inputs):
    (W3, W4, W5), bpack = _prep_weights(inputs)
    w345 = np.concatenate([W3, W4, W5], axis=1).astype(BF)
    wpk, fpk = [], []
    for h in range(2):
        W1, W2a, W2c, W2d, W2e = _prep_conv12_weights(inputs, h)
        wpk.append((W1, W2a.astype(BF),
                    np.concatenate([W2c, W2d, W2e], axis=1).astype(BF)))
        fpk.append(np.concatenate(
            [bpack, _prep_fcw(inputs["fcw"], h)], axis=1))
    xr = np.asarray(inputs["xr"], np.float32)
    xi = np.asarray(inputs["xi"], np.float32)
    in_maps = []
    for core in range(8):
        b, h = core // 2, core % 2
        xr_b, xi_b = xr[b, 0], xi[b, 0]
        if h == 1:
            xr_b, xi_b = _mirror_x(xr_b), _mirror_x(xi_b)
        in_maps.append({
            "x1": _prep_x1(xr_b, xi_b),
            "w1": wpk[h][0], "w2a": wpk[h][1], "w2cd": wpk[h][2],
            "w345": w345, "fpk": fpk[h],
        })
    return in_maps


def kernel(**inputs):
    if "nc" not in _CACHE:
        _CACHE["nc"] = _build_nc()
    nc = _CACHE["nc"]

    in_maps = _make_in_maps(inputs)

    res = run_bass_kernel_spmd(nc, in_maps, core_ids=list(range(8)))

    fcb = np.asarray(inputs["fcb"], np.float32)
    yr = np.zeros((NB, 64, 1), np.float32)
    yi = np.zeros((NB, 64, 1), np.float32)
    for b in range(NB):
        p0 = res.results[2 * b]["out"]
        p1 = res.results[2 * b + 1]["out"]
        s = p0 + p1
        yr[b] = s[0:64] + fcb[0]
        yi[b] = s[64:128]
    return np.stack([yr, yi]).astype(np.float32)
